# revision 22
# baseline (speedup 1.0000x reference)
"""MimiAttention (sliding-window, RoPE) Bass kernel for 8 TRN2 cores.

Sharding: core c -> (b = c//2, seq-half = c%2). Each core computes its
1024 output rows end-to-end (QKV proj + RoPE + banded attention + out
proj); kv halo of 256 rows is zero-padded for the first half.

All activations/weights are bf16 (host-cast); PSUM accumulation fp32.

v2 design notes:
 - Score chunks are trimmed to the live band: per 256-q superblock the
   4 kv chunks pack as [kc1(256) kc2(256) kc0(128) kc3(128)] = 768
   psum cols (25% less PE/ACT/AV work than the naive 1024).
 - Scores run start=True per segment, mask bias accumulates AFTER via
   identity matmuls (2 per head: N=512+N=256).
 - Head pairs (even rows 0-63 / odd rows 64-127 of the packed qT/kT
   tiles) interleave their K=64 score matmuls -> row-tiled concurrency
   on the PE array.
 - Softmax normalization: reciprocal of the ones-column sums row, PE
   broadcast matmul (indicator lhsT) expands recA/recB to [128,512],
   ACT evacuates to bf16, and the pav->outT PSUM evacuation is a fused
   DVE multiply. No gpsimd partition_broadcast, no tiny DMAs.
 - qp-major attention ordering so out-proj for the first 512 q cols
   overlaps attention on the second half.
"""

import os
import numpy as np
import ml_dtypes

BF16 = ml_dtypes.bfloat16

B, S, HID = 4, 2048, 512
NH, HD = 8, 64
SW = 250
THETA = 10000.0
SCALING = 1.0 / np.sqrt(HD)
N_CORES = 8
HALO = 256
SKV = HALO + S // 2   # 1280 kv rows per core
SQ = S // 2           # 1024 q rows per core
QSB = 256             # q super-block
NQSB = SQ // QSB      # 4
NEG = -float(2 ** 30)

# packed score segments per 256-q superblock: (kc, psum col off, width, q off)
SEGS = [(1, 0, 256, 0), (2, 256, 256, 0), (0, 512, 128, 0), (3, 640, 128, 128)]
SEGW = 768

_cache = {}

LAST_RESULT = None


def _build_nc():
    import concourse.bacc as bacc
    import concourse.mybir as mybir
    from concourse import tile

    f32 = mybir.dt.float32
    bf16 = mybir.dt.bfloat16
    EXP = mybir.ActivationFunctionType.Exp

    nc = bacc.Bacc("TRN2", target_bir_lowering=False, debug=False,
                   num_devices=N_CORES)

    hsT = nc.dram_tensor("hsT", [HID, SKV], bf16, kind="ExternalInput").ap()
    wqT = nc.dram_tensor("wqT", [HID, HID], bf16, kind="ExternalInput").ap()
    wkT = nc.dram_tensor("wkT", [HID, HID], bf16, kind="ExternalInput").ap()
    wvT = nc.dram_tensor("wvT", [HID, HID], bf16, kind="ExternalInput").ap()
    woT = nc.dram_tensor("woT", [HID, HID], bf16, kind="ExternalInput").ap()
    cosT = nc.dram_tensor("cosT", [128, SKV], bf16, kind="ExternalInput").ap()
    sinT = nc.dram_tensor("sinT", [128, SKV], bf16, kind="ExternalInput").ap()
    maskT = nc.dram_tensor("maskT", [128, 2 * SEGW], bf16, kind="ExternalInput").ap()
    onesin = nc.dram_tensor("onesin", [128, NH], bf16, kind="ExternalInput").ap()
    out = nc.dram_tensor("out", [SQ, HID], bf16, kind="ExternalOutput").ap()

    with tile.TileContext(nc) as tc:
        with (
            tc.tile_pool(name="w", bufs=1) as wp,       # persistent weights/consts
            tc.tile_pool(name="act", bufs=1) as ap_,    # persistent activations
            tc.tile_pool(name="ex", bufs=2) as exp_p,   # exp probs
            tc.tile_pool(name="no", bufs=2) as np_,     # normalize staging
            tc.tile_pool(name="oo", bufs=2) as op_,     # out staging
            tc.tile_pool(name="psp", bufs=2, space="PSUM") as psp,   # proj/outproj/recb
            tc.tile_pool(name="pss", bufs=2, space="PSUM") as pss,   # scoresT
            tc.tile_pool(name="psa", bufs=2, space="PSUM") as psa,   # AV
        ):
            # ---- persistent tiles (loads emitted in dependency order below)
            wv_sb = [wp.tile([128, HID], bf16, tag=f"wv{kc}", name=f"wv{kc}")
                     for kc in range(4)]
            wo_sb = [wp.tile([128, HID], bf16, tag=f"wo{kc}", name=f"wo{kc}")
                     for kc in range(4)]
            cos_sb = wp.tile([128, SKV], bf16, tag="cos")
            sin_sb = wp.tile([128, SKV], bf16, tag="sin")
            mask_sb = wp.tile([128, 2 * SEGW], bf16, tag="mask")

            qT_sb = [ap_.tile([128, SQ], bf16, tag=f"qT{t}", name=f"qT{t}") for t in range(4)]
            kT_sb = [ap_.tile([128, SKV], bf16, tag=f"kT{t}", name=f"kT{t}") for t in range(4)]
            v_sb = [ap_.tile([128, NH * (HD + 1)], bf16, tag=f"v{st}", name=f"v{st}")
                    for st in range(SKV // 128)]
            outT_sb = [ap_.tile([128, SQ], bf16, tag=f"oT{t}", name=f"oT{t}") for t in range(4)]

            # ---- attention: one head pair (2m, 2m+1), one 256-q superblock
            def attn_pair_qsb(hm, qsb):
                q0 = qsb * QSB
                bv = 0 if qsb == 0 else 1
                pss_pair = []
                for r0 in (0, 64):
                    pss_pair.append(pss.tile([128, 1024], f32, tag="pssc",
                                             name=f"ps{hm}_{qsb}_{r0}"))
                # row-tiled score matmuls: even head rows 0-63, odd rows 64-127.
                # start=True only on the first matmul touching each PSUM bank
                # (start marks the whole 2KB bank pending-zero).
                for (kc, off, w, qoff) in SEGS:
                    kv0 = q0 + kc * 128
                    for i, r0 in enumerate((0, 64)):
                        nc.tensor.matmul(
                            pss_pair[i][:, off:off + w],
                            kT_sb[hm][r0:r0 + 64, kv0:kv0 + 128],
                            qT_sb[hm][r0:r0 + 64, q0 + qoff:q0 + qoff + w],
                            start=(off in (0, 512)), stop=(off in (256, 640)),
                            skip_group_check=True,
                        )
                # exp then multiplicative 0/1 mask on DVE (no PE bias matmuls;
                # exp can't overflow: raw scores stay well within bf16 range)
                ets = []
                for i, r0 in enumerate((0, 64)):
                    et = exp_p.tile([128, SEGW], bf16, tag="expT",
                                    name=f"et{hm}_{qsb}_{r0}")
                    nc.scalar.activation(et[:], pss_pair[i][:, 0:SEGW], EXP)
                    nc.vector.tensor_mul(
                        et[:], et[:], mask_sb[:, bv * SEGW:(bv + 1) * SEGW])
                    ets.append(et)
                return ets

            def av_qsb(h, qsb, et, pav):
                # pav is one 2KB bank covering both qsb halves: only the very
                # first matmul (even qsb, seg 0) sets start
                base = (qsb % 2) * 256
                for si, (kc, off, w, qoff) in enumerate(SEGS):
                    st = 2 * qsb + kc
                    nc.tensor.matmul(
                        pav[:, base + qoff: base + qoff + w],
                        v_sb[st][:, h * 65:(h + 1) * 65],
                        et[:, off:off + w],
                        start=(si == 0 and qsb % 2 == 0), stop=(si >= 2),
                        skip_group_check=True,
                    )

            # ---- normalization: fused into pav -> outT evacuation.
            # sums row (PSUM partition 64) -> SBUF row 0 copy, reciprocal in
            # SBUF, gpsimd partition-broadcast, then the PSUM evacuation IS
            # the normalize multiply (all constructs HW-proven).
            def norm_pair(hm, qp, pavA, pavB):
                q0 = qp * 512
                srow = np_.tile([33, 512], f32, tag="srow", name=f"sr{hm}{qp}")
                nc.vector.tensor_copy(srow[0:1, :], pavA[64:65, :])
                nc.vector.tensor_copy(srow[32:33, :], pavB[64:65, :])
                rrow = np_.tile([33, 512], f32, tag="rrow", name=f"rr{hm}{qp}")
                nc.vector.reciprocal_approx_fast(rrow[:], srow[:])
                for pav, r0 in ((pavA, 0), (pavB, 64)):
                    rb = np_.tile([128, 512], f32, tag="rb", name=f"rb{hm}{qp}{r0}")
                    nc.gpsimd.partition_broadcast(rb[:], rrow[(r0 // 2):(r0 // 2) + 1, :])
                    nc.vector.tensor_mul(
                        outT_sb[hm][r0:r0 + 64, q0:q0 + 512], pav[0:64, :],
                        rb[r0:r0 + 64, :])

            # one head-pair, one qp half (2 superblocks + norm)
            def attn_pair_qp(hm, qp):
                pavA = psa.tile([65, 512], f32, tag="pav", name=f"pavA{hm}_{qp}")
                pavB = psa.tile([65, 512], f32, tag="pav", name=f"pavB{hm}_{qp}")
                for qsb in (2 * qp, 2 * qp + 1):
                    etA, etB = attn_pair_qsb(hm, qsb)
                    av_qsb(2 * hm, qsb, etA, pavA)
                    av_qsb(2 * hm + 1, qsb, etB, pavB)
                norm_pair(hm, qp, pavA, pavB)

            def outproj_qp(qp):
                for ot in range(4 * qp, 4 * qp + 4):
                    po = psp.tile([128, HID], f32, tag="psproj", name=f"po{ot}")
                    for dc in range(4):
                        nc.tensor.matmul(
                            po[:],
                            outT_sb[dc][:, ot * 128:(ot + 1) * 128],
                            wo_sb[dc][:],
                            start=(dc == 0), stop=(dc == 3),
                        )
                    ob = op_.tile([128, HID], bf16, tag="ob")
                    nc.scalar.copy(ob[:], po[:])
                    nc.sync.dma_start(out=out[ot * 128:(ot + 1) * 128, :], in_=ob[:])

            # ---- projection scope ----
            with (
                tc.tile_pool(name="pw", bufs=1) as pwp,
                tc.tile_pool(name="tmp", bufs=2) as tp,
            ):
                hsT_sb = [pwp.tile([128, SKV], bf16, tag=f"hsT{kc}", name=f"hsT{kc}")
                          for kc in range(4)]

                def load_w(dram):
                    return [pwp.tile([128, HID], bf16, tag=f"w{dram.name}{kc}",
                                     name=f"w{dram.name}{kc}") for kc in range(4)]

                wq_sb = load_w(wqT)
                wk_sb = load_w(wkT)
                # dependency-ordered input DMAs: proj needs wq/hsT first
                for kc in range(4):
                    nc.sync.dma_start(out=wq_sb[kc][:], in_=wqT[kc * 128:(kc + 1) * 128, :])
                    nc.sync.dma_start(out=hsT_sb[kc][:], in_=hsT[kc * 128:(kc + 1) * 128, :])
                nc.sync.dma_start(out=cos_sb[:], in_=cosT[:])
                nc.sync.dma_start(out=sin_sb[:], in_=sinT[:])
                for kc in range(4):
                    nc.sync.dma_start(out=wk_sb[kc][:], in_=wkT[kc * 128:(kc + 1) * 128, :])
                for kc in range(4):
                    nc.sync.dma_start(out=wv_sb[kc][:], in_=wvT[kc * 128:(kc + 1) * 128, :])
                nc.sync.dma_start(out=mask_sb[:], in_=maskT[:])
                for kc in range(4):
                    nc.sync.dma_start(out=wo_sb[kc][:], in_=woT[kc * 128:(kc + 1) * 128, :])

                def proj_rope(w_tiles, out_tiles, col0, ncols, dma_eng, pairs):
                    # per pair: matmul+rope all s-chunks into full-width perm
                    # staging, then 8 contiguous sb->sb DMAs rearrange rows
                    # into head-major tiles
                    for (a, b2) in pairs:
                        stA = tp.tile([128, SKV], bf16, tag="stA", name=f"stA{a}", bufs=1)
                        stB = tp.tile([128, SKV], bf16, tag="stB", name=f"stB{a}", bufs=1)
                        sc = 0
                        while sc < ncols:
                            w = min(512, ncols - sc)
                            c = cos_sb[:, col0 + sc: col0 + sc + w]
                            s = sin_sb[:, col0 + sc: col0 + sc + w]
                            ps = {}
                            for t in (a, b2):
                                p = psp.tile([128, 512], f32, tag="psproj", name=f"pj{t}")
                                for kc in range(4):
                                    nc.tensor.matmul(
                                        p[:, :w],
                                        w_tiles[kc][:, t * 128:(t + 1) * 128],
                                        hsT_sb[kc][:, col0 + sc: col0 + sc + w],
                                        start=(kc == 0), stop=(kc == 3),
                                    )
                                ps[t] = p
                            # evac PSUM via ACT so muls run all-SBUF bf16
                            eA = tp.tile([128, 512], bf16, tag="eA", bufs=1)
                            eB = tp.tile([128, 512], bf16, tag="eB", bufs=1)
                            nc.scalar.copy(eA[:, :w], ps[a][:, :w])
                            nc.scalar.copy(eB[:, :w], ps[b2][:, :w])
                            m1 = tp.tile([128, 512], bf16, tag="m1")
                            m2 = tp.tile([128, 512], bf16, tag="m2")
                            m3 = tp.tile([128, 512], bf16, tag="m3")
                            m4 = tp.tile([128, 512], bf16, tag="m4")
                            nc.vector.tensor_mul(m1[:, :w], eA[:, :w], c)
                            nc.vector.tensor_mul(m2[:, :w], eB[:, :w], s)
                            nc.gpsimd.tensor_mul(m3[:, :w], eB[:, :w], c)
                            nc.vector.tensor_mul(m4[:, :w], eA[:, :w], s)
                            nc.vector.tensor_sub(stA[:, sc:sc + w], m1[:, :w], m2[:, :w])
                            nc.vector.tensor_add(stB[:, sc:sc + w], m3[:, :w], m4[:, :w])
                            sc += w
                        for g in range(4):
                            head = (a % 2) * 4 + g
                            hm, r1 = head // 2, (head % 2) * 64
                            gs = slice(g * 32, g * 32 + 32)
                            dma_eng.dma_start(out=out_tiles[hm][r1:r1 + 32, 0:ncols],
                                              in_=stA[gs, 0:ncols])
                            dma_eng.dma_start(out=out_tiles[hm][r1 + 32:r1 + 64, 0:ncols],
                                              in_=stB[gs, 0:ncols])

                def proj_pair(pair):
                    proj_rope(wq_sb, qT_sb, HALO, SQ, nc.sync, [pair])
                    proj_rope(wk_sb, kT_sb, 0, SKV, nc.scalar, [pair])

                # ---- V in natural layout with ones columns (65 per head) ----
                PAIR0, PAIR1 = (0, 2), (1, 3)

                def emit_v():
                    for st in range(SKV // 128):
                        vt = v_sb[st]
                        ones_dst = vt[:].rearrange("p (h c) -> p h c", h=NH, c=HD + 1)[:, :, HD:HD + 1]
                        nc.sync.dma_start(out=ones_dst,
                                          in_=onesin[:].rearrange("p (h c) -> p h c", h=NH, c=1))
                        p = psp.tile([128, 512], f32, tag="psproj")
                        for kc in range(4):
                            nc.tensor.matmul(
                                p[:],
                                hsT_sb[kc][:, st * 128:(st + 1) * 128],
                                wv_sb[kc][:],
                                start=(kc == 0), stop=(kc == 3),
                            )
                        dstv = vt[:].rearrange("p (h c) -> p h c", h=NH, c=HD + 1)[:, :, 0:HD]
                        nc.vector.tensor_copy(dstv, p[:].rearrange("p (h c) -> p h c", h=NH, c=HD))

                # interleaved emission: qp-major so outproj(qp0) overlaps
                # attention(qp1); PAIR1 proj overlaps early attention.
                proj_pair(PAIR0)
                emit_v()
                attn_pair_qp(0, 0)
                attn_pair_qp(1, 0)
                proj_pair(PAIR1)
                attn_pair_qp(2, 0)
                attn_pair_qp(3, 0)
                attn_pair_qp(0, 1)
                outproj_qp(0)
                for hm in range(1, 4):
                    attn_pair_qp(hm, 1)
            outproj_qp(1)

    nc.compile()
    return nc


def _perm():
    p = np.empty(512, np.int64)
    i = 0
    for t in range(4):
        for g in range(4):
            for j in range(32):
                head = (t % 2) * 4 + g
                p[i] = head * 64 + (t // 2) * 32 + j
                i += 1
    return p


def _mask_packed(qstart, v):
    """Packed 0/1 mask [128, 768] for superblock variant v (0: qsb==0,
    1: qsb>=1), for the core whose q rows start at qstart."""
    out = np.empty((128, SEGW), np.float32)
    p = np.arange(128)[:, None]
    for (kc, off, w, qoff) in SEGS:
        i = qoff + np.arange(w)[None, :]
        kv_abs = qstart + v * QSB + kc * 128 + p - 256
        q_abs = qstart + v * QSB + i
        ok = (q_abs >= kv_abs) & (q_abs - kv_abs <= SW) & (kv_abs >= 0)
        out[:, off:off + w] = np.where(ok, 1.0, 0.0)
    return out


def _host_inputs(hidden_states, position_ids, Wq, Wk, Wv, Wo):
    hs = np.asarray(hidden_states, np.float32)
    pos = np.asarray(position_ids, np.float32)
    perm = _perm()
    wq_h = np.ascontiguousarray((np.asarray(Wq, np.float32) * SCALING)[perm].T).astype(BF16)
    wk_h = np.ascontiguousarray(np.asarray(Wk, np.float32)[perm].T).astype(BF16)
    wv_h = np.ascontiguousarray(np.asarray(Wv, np.float32).T).astype(BF16)
    wo_h = np.ascontiguousarray(np.asarray(Wo, np.float32).T).astype(BF16)
    inv = (THETA ** (-np.arange(32, dtype=np.float32) / 32.0)).astype(np.float32)
    ones8 = np.ones((128, NH), np.float32).astype(BF16)

    in_maps = []
    for c in range(N_CORES):
        b, half = c // 2, c % 2
        qstart = half * SQ
        lo = qstart - HALO
        hsT = np.zeros((HID, SKV), np.float32)
        src_lo = max(lo, 0)
        hsT[:, src_lo - lo:] = hs[b, src_lo:qstart + SQ, :].T
        pbuf = np.zeros(SKV, np.float32)
        pbuf[src_lo - lo:] = pos[b, src_lo:qstart + SQ]
        fr = inv[:, None] * pbuf[None, :]
        cosT = np.tile(np.cos(fr), (4, 1)).astype(BF16)
        sinT = np.tile(np.sin(fr), (4, 1)).astype(BF16)
        maskT = np.concatenate(
            [_mask_packed(qstart, 0), _mask_packed(qstart, 1)], axis=1)
        in_maps.append({
            "hsT": np.ascontiguousarray(hsT).astype(BF16),
            "wqT": wq_h, "wkT": wk_h, "wvT": wv_h, "woT": wo_h,
            "cosT": cosT, "sinT": sinT, "maskT": maskT.astype(BF16),
            "onesin": ones8,
        })
    return in_maps


def kernel(hidden_states, position_ids, Wq, Wk, Wv, Wo):
    global LAST_RESULT
    from concourse.bass_utils import run_bass_kernel_spmd

    if "nc" not in _cache:
        _cache["nc"] = _build_nc()
    nc = _cache["nc"]

    in_maps = _host_inputs(hidden_states, position_ids, Wq, Wk, Wv, Wo)
    trace = bool(os.environ.get("KERNEL_TRACE"))
    kw = {}
    if trace:
        kw = dict(trace=True, tmpdir=os.environ.get("KERNEL_TRACE_DIR") or None)
    res = run_bass_kernel_spmd(nc, in_maps, list(range(N_CORES)), **kw)
    LAST_RESULT = res

    out = np.empty((B, S, HID), np.float32)
    for c in range(N_CORES):
        b, half = c // 2, c % 2
        out[b, half * SQ:(half + 1) * SQ, :] = np.asarray(
            res.results[c]["out"], dtype=np.float32)
    return out


# revision 31
# speedup vs baseline: 1.0767x; 1.0767x over previous
"""MimiAttention (sliding-window, RoPE) Bass kernel for 8 TRN2 cores.

Sharding: core c -> (b = c//2, seq-half = c%2). Each core computes its
1024 output rows end-to-end (QKV proj + RoPE + banded attention + out
proj); kv halo of 256 rows is zero-padded for the first half.

All activations/weights are bf16 (host-cast); PSUM accumulation fp32.

v2 design notes:
 - Score chunks are trimmed to the live band: per 256-q superblock the
   4 kv chunks pack as [kc1(256) kc2(256) kc0(128) kc3(128)] = 768
   psum cols (25% less PE/ACT/AV work than the naive 1024).
 - Scores run start=True per segment, mask bias accumulates AFTER via
   identity matmuls (2 per head: N=512+N=256).
 - Head pairs (even rows 0-63 / odd rows 64-127 of the packed qT/kT
   tiles) interleave their K=64 score matmuls -> row-tiled concurrency
   on the PE array.
 - Softmax normalization: reciprocal of the ones-column sums row, PE
   broadcast matmul (indicator lhsT) expands recA/recB to [128,512],
   ACT evacuates to bf16, and the pav->outT PSUM evacuation is a fused
   DVE multiply. No gpsimd partition_broadcast, no tiny DMAs.
 - qp-major attention ordering so out-proj for the first 512 q cols
   overlaps attention on the second half.
"""

import os
import numpy as np
import ml_dtypes

BF16 = ml_dtypes.bfloat16

B, S, HID = 4, 2048, 512
NH, HD = 8, 64
SW = 250
THETA = 10000.0
SCALING = 1.0 / np.sqrt(HD)
N_CORES = 8
HALO = 256
SKV = HALO + S // 2   # 1280 kv rows per core
SQ = S // 2           # 1024 q rows per core
QSB = 256             # q super-block
NQSB = SQ // QSB      # 4
NEG = -float(2 ** 30)

# packed score segments per 256-q superblock: (kc, psum col off, width, q off)
SEGS = [(1, 0, 256, 0), (2, 256, 256, 0), (0, 512, 128, 0), (3, 640, 128, 128)]
SEGW = 768

_cache = {}

LAST_RESULT = None


def _build_nc():
    import concourse.bacc as bacc
    import concourse.mybir as mybir
    from concourse import tile

    f32 = mybir.dt.float32
    bf16 = mybir.dt.bfloat16
    EXP = mybir.ActivationFunctionType.Exp

    nc = bacc.Bacc("TRN2", target_bir_lowering=False, debug=False,
                   num_devices=N_CORES)

    hsT = nc.dram_tensor("hsT", [HID, SKV], bf16, kind="ExternalInput").ap()
    wqT = nc.dram_tensor("wqT", [HID, HID], bf16, kind="ExternalInput").ap()
    wkT = nc.dram_tensor("wkT", [HID, HID], bf16, kind="ExternalInput").ap()
    wvT = nc.dram_tensor("wvT", [HID, HID], bf16, kind="ExternalInput").ap()
    woT = nc.dram_tensor("woT", [HID, HID], bf16, kind="ExternalInput").ap()
    cosT = nc.dram_tensor("cosT", [128, SKV], bf16, kind="ExternalInput").ap()
    sinT = nc.dram_tensor("sinT", [128, SKV], bf16, kind="ExternalInput").ap()
    maskT = nc.dram_tensor("maskT", [128, 2 * SEGW], bf16, kind="ExternalInput").ap()
    onesin = nc.dram_tensor("onesin", [128, NH], bf16, kind="ExternalInput").ap()
    out = nc.dram_tensor("out", [SQ, HID], bf16, kind="ExternalOutput").ap()

    with tile.TileContext(nc) as tc:
        with (
            tc.tile_pool(name="w", bufs=1) as wp,       # persistent weights/consts
            tc.tile_pool(name="act", bufs=1) as ap_,    # persistent activations
            tc.tile_pool(name="ex", bufs=3) as exp_p,   # exp probs
            tc.tile_pool(name="no", bufs=2) as np_,     # normalize staging
            tc.tile_pool(name="oo", bufs=2) as op_,     # out staging
            tc.tile_pool(name="psp", bufs=2, space="PSUM") as psp,   # proj/outproj/recb
            tc.tile_pool(name="pss", bufs=2, space="PSUM") as pss,   # scoresT
            tc.tile_pool(name="psa", bufs=2, space="PSUM") as psa,   # AV
        ):
            # ---- persistent tiles (loads emitted in dependency order below)
            wv_sb = [wp.tile([128, HID], bf16, tag=f"wv{kc}", name=f"wv{kc}")
                     for kc in range(4)]
            wo_sb = [wp.tile([128, HID], bf16, tag=f"wo{kc}", name=f"wo{kc}")
                     for kc in range(4)]
            cos_sb = wp.tile([128, SKV], bf16, tag="cos")
            sin_sb = wp.tile([128, SKV], bf16, tag="sin")
            mask_sb = wp.tile([128, 2 * SEGW], bf16, tag="mask")

            qT_sb = [ap_.tile([128, SQ], bf16, tag=f"qT{t}", name=f"qT{t}") for t in range(4)]
            kT_sb = [ap_.tile([128, SKV], bf16, tag=f"kT{t}", name=f"kT{t}") for t in range(4)]
            v_sb = [ap_.tile([128, NH * (HD + 1)], bf16, tag=f"v{st}", name=f"v{st}")
                    for st in range(SKV // 128)]
            outT_sb = [ap_.tile([128, SQ], bf16, tag=f"oT{t}", name=f"oT{t}") for t in range(4)]

            # ---- attention: one head, one 256-q superblock
            def attn_head_qsb(h, qsb):
                hm, r0 = h // 2, (h % 2) * 64
                q0 = qsb * QSB
                bv = 0 if qsb == 0 else 1
                pssc = pss.tile([128, SEGW], f32, tag="pssc",
                                name=f"ps{h}_{qsb}")
                # start=True only on the first matmul touching each PSUM bank
                # (start marks the whole 2KB bank pending-zero).
                # pool tiles are bank-aligned: banks start at cols 0 and 512.
                # start=True exactly on the first matmul touching each bank
                # (start marks the whole 2KB bank pending-zero).
                for (kc, off, w, qoff) in SEGS:
                    kv0 = q0 + kc * 128
                    nc.tensor.matmul(
                        pssc[:, off:off + w],
                        kT_sb[hm][r0:r0 + 64, kv0:kv0 + 128],
                        qT_sb[hm][r0:r0 + 64, q0 + qoff:q0 + qoff + w],
                        start=(off in (0, 512)), stop=(off in (256, 640)),
                        skip_group_check=True,
                    )
                # exp then multiplicative 0/1 mask on DVE (no PE bias matmuls;
                # exp can't overflow: raw scores stay well within bf16 range)
                et = exp_p.tile([128, SEGW], bf16, tag="expT",
                                name=f"et{h}_{qsb}")
                nc.scalar.activation(et[:], pssc[:], EXP)
                nc.vector.tensor_mul(
                    et[:], et[:], mask_sb[:, bv * SEGW:(bv + 1) * SEGW])
                return et

            def av_qsb(h, qsb, et, pav):
                # pav is one 2KB bank covering both qsb halves: only the very
                # first matmul (even qsb, seg 0) sets start
                base = (qsb % 2) * 256
                for si, (kc, off, w, qoff) in enumerate(SEGS):
                    st = 2 * qsb + kc
                    nc.tensor.matmul(
                        pav[:, base + qoff: base + qoff + w],
                        v_sb[st][:, h * 65:(h + 1) * 65],
                        et[:, off:off + w],
                        start=(si == 0 and qsb % 2 == 0), stop=(si >= 2),
                        skip_group_check=True,
                    )

            # ---- normalization: fused into pav -> outT evacuation.
            # sums row (PSUM partition 64) -> SBUF row 0 copy, reciprocal in
            # SBUF, gpsimd partition-broadcast, then the PSUM evacuation IS
            # the normalize multiply (all constructs HW-proven).
            def norm_head(h, qp, pav):
                hm, r0 = h // 2, (h % 2) * 64
                q0 = qp * 512
                srow = np_.tile([1, 512], f32, tag="srow", name=f"sr{h}{qp}")
                nc.vector.tensor_copy(srow[:], pav[64:65, :])
                rrow = np_.tile([1, 512], f32, tag="rrow", name=f"rr{h}{qp}")
                nc.vector.reciprocal_approx_fast(rrow[:], srow[:])
                rb = np_.tile([128, 512], f32, tag="rb", name=f"rb{h}{qp}")
                nc.gpsimd.partition_broadcast(rb[:], rrow[:])
                nc.vector.tensor_mul(
                    outT_sb[hm][r0:r0 + 64, q0:q0 + 512], pav[0:64, :],
                    rb[r0:r0 + 64, :])

            # software-pipelined attention over (head, qsb) units: the next
            # unit's score matmuls are emitted before this unit's AV so the
            # PE never stalls on the exp+mask chain.
            pend = []   # (h, qsb, et, pav)
            pavs = {}

            def attn_drain():
                while pend:
                    h, qsb, et, pav = pend.pop(0)
                    av_qsb(h, qsb, et, pav)
                    if qsb % 2 == 1:
                        norm_head(h, qsb // 2, pav)

            def attn_unit(h, qsb):
                if qsb % 2 == 0:
                    pavs[h] = psa.tile([65, 512], f32, tag="pav",
                                       name=f"pav{h}_{qsb // 2}")
                et = attn_head_qsb(h, qsb)
                pend.append((h, qsb, et, pavs[h]))
                if len(pend) > 1:
                    h2, qsb2, et2, pav2 = pend.pop(0)
                    av_qsb(h2, qsb2, et2, pav2)
                    if qsb2 % 2 == 1:
                        norm_head(h2, qsb2 // 2, pav2)

            def attn_pair_qp(hm, qp):
                for qsb in (2 * qp, 2 * qp + 1):
                    for h in (2 * hm, 2 * hm + 1):
                        attn_unit(h, qsb)

            def outproj_qp(qp):
                for ot in range(4 * qp, 4 * qp + 4):
                    po = psp.tile([128, HID], f32, tag="psproj", name=f"po{ot}")
                    for dc in range(4):
                        nc.tensor.matmul(
                            po[:],
                            outT_sb[dc][:, ot * 128:(ot + 1) * 128],
                            wo_sb[dc][:],
                            start=(dc == 0), stop=(dc == 3),
                        )
                    ob = op_.tile([128, HID], bf16, tag="ob")
                    nc.scalar.copy(ob[:], po[:])
                    nc.sync.dma_start(out=out[ot * 128:(ot + 1) * 128, :], in_=ob[:])

            # ---- projection scope ----
            with (
                tc.tile_pool(name="pw", bufs=1) as pwp,
                tc.tile_pool(name="tmp", bufs=2) as tp,
            ):
                hsT_sb = [pwp.tile([128, SKV], bf16, tag=f"hsT{kc}", name=f"hsT{kc}")
                          for kc in range(4)]

                def load_w(dram):
                    return [pwp.tile([128, HID], bf16, tag=f"w{dram.name}{kc}",
                                     name=f"w{dram.name}{kc}") for kc in range(4)]

                wq_sb = load_w(wqT)
                wk_sb = load_w(wkT)
                # dependency-ordered input DMAs: proj needs wq/hsT first
                for kc in range(4):
                    nc.sync.dma_start(out=wq_sb[kc][:], in_=wqT[kc * 128:(kc + 1) * 128, :])
                    nc.sync.dma_start(out=hsT_sb[kc][:], in_=hsT[kc * 128:(kc + 1) * 128, :])
                nc.sync.dma_start(out=cos_sb[:], in_=cosT[:])
                nc.sync.dma_start(out=sin_sb[:], in_=sinT[:])
                for kc in range(4):
                    nc.sync.dma_start(out=wk_sb[kc][:], in_=wkT[kc * 128:(kc + 1) * 128, :])
                for kc in range(4):
                    nc.sync.dma_start(out=wv_sb[kc][:], in_=wvT[kc * 128:(kc + 1) * 128, :])
                nc.sync.dma_start(out=mask_sb[:], in_=maskT[:])
                for kc in range(4):
                    nc.sync.dma_start(out=wo_sb[kc][:], in_=woT[kc * 128:(kc + 1) * 128, :])

                def proj_rope(w_tiles, out_tiles, col0, ncols, dma_eng, pairs):
                    # per pair: matmul+rope all s-chunks into full-width perm
                    # staging, then 8 contiguous sb->sb DMAs rearrange rows
                    # into head-major tiles
                    for (a, b2) in pairs:
                        stA = tp.tile([128, SKV], bf16, tag="stA", name=f"stA{a}", bufs=1)
                        stB = tp.tile([128, SKV], bf16, tag="stB", name=f"stB{a}", bufs=1)
                        sc = 0
                        while sc < ncols:
                            w = min(512, ncols - sc)
                            c = cos_sb[:, col0 + sc: col0 + sc + w]
                            s = sin_sb[:, col0 + sc: col0 + sc + w]
                            ps = {}
                            for t in (a, b2):
                                p = psp.tile([128, 512], f32, tag="psproj", name=f"pj{t}")
                                for kc in range(4):
                                    nc.tensor.matmul(
                                        p[:, :w],
                                        w_tiles[kc][:, t * 128:(t + 1) * 128],
                                        hsT_sb[kc][:, col0 + sc: col0 + sc + w],
                                        start=(kc == 0), stop=(kc == 3),
                                    )
                                ps[t] = p
                            # evac PSUM via ACT so muls run all-SBUF bf16
                            eA = tp.tile([128, 512], bf16, tag="eA", bufs=1)
                            eB = tp.tile([128, 512], bf16, tag="eB", bufs=1)
                            nc.scalar.copy(eA[:, :w], ps[a][:, :w])
                            nc.scalar.copy(eB[:, :w], ps[b2][:, :w])
                            m1 = tp.tile([128, 512], bf16, tag="m1")
                            m2 = tp.tile([128, 512], bf16, tag="m2")
                            m3 = tp.tile([128, 512], bf16, tag="m3")
                            m4 = tp.tile([128, 512], bf16, tag="m4")
                            nc.vector.tensor_mul(m1[:, :w], eA[:, :w], c)
                            nc.vector.tensor_mul(m2[:, :w], eB[:, :w], s)
                            nc.gpsimd.tensor_mul(m3[:, :w], eB[:, :w], c)
                            nc.vector.tensor_mul(m4[:, :w], eA[:, :w], s)
                            nc.vector.tensor_sub(stA[:, sc:sc + w], m1[:, :w], m2[:, :w])
                            nc.vector.tensor_add(stB[:, sc:sc + w], m3[:, :w], m4[:, :w])
                            sc += w
                        for g in range(4):
                            head = (a % 2) * 4 + g
                            hm, r1 = head // 2, (head % 2) * 64
                            gs = slice(g * 32, g * 32 + 32)
                            dma_eng.dma_start(out=out_tiles[hm][r1:r1 + 32, 0:ncols],
                                              in_=stA[gs, 0:ncols])
                            dma_eng.dma_start(out=out_tiles[hm][r1 + 32:r1 + 64, 0:ncols],
                                              in_=stB[gs, 0:ncols])

                def proj_pair(pair):
                    proj_rope(wq_sb, qT_sb, HALO, SQ, nc.sync, [pair])
                    proj_rope(wk_sb, kT_sb, 0, SKV, nc.scalar, [pair])

                # ---- V in natural layout with ones columns (65 per head) ----
                PAIR0, PAIR1 = (0, 2), (1, 3)

                def emit_v():
                    for st in range(SKV // 128):
                        vt = v_sb[st]
                        ones_dst = vt[:].rearrange("p (h c) -> p h c", h=NH, c=HD + 1)[:, :, HD:HD + 1]
                        nc.sync.dma_start(out=ones_dst,
                                          in_=onesin[:].rearrange("p (h c) -> p h c", h=NH, c=1))
                        p = psp.tile([128, 512], f32, tag="psproj")
                        for kc in range(4):
                            nc.tensor.matmul(
                                p[:],
                                hsT_sb[kc][:, st * 128:(st + 1) * 128],
                                wv_sb[kc][:],
                                start=(kc == 0), stop=(kc == 3),
                            )
                        dstv = vt[:].rearrange("p (h c) -> p h c", h=NH, c=HD + 1)[:, :, 0:HD]
                        nc.vector.tensor_copy(dstv, p[:].rearrange("p (h c) -> p h c", h=NH, c=HD))

                # interleaved emission: qp-major so outproj(qp0) overlaps
                # attention(qp1); PAIR1 proj overlaps early attention.
                proj_pair(PAIR0)
                emit_v()
                attn_pair_qp(0, 0)
                attn_pair_qp(1, 0)
                proj_pair(PAIR1)
                attn_pair_qp(2, 0)
                attn_pair_qp(3, 0)
                attn_pair_qp(0, 1)
                outproj_qp(0)
                for hm in range(1, 4):
                    attn_pair_qp(hm, 1)
                attn_drain()
            outproj_qp(1)

    nc.compile()
    return nc


def _perm():
    p = np.empty(512, np.int64)
    i = 0
    for t in range(4):
        for g in range(4):
            for j in range(32):
                head = (t % 2) * 4 + g
                p[i] = head * 64 + (t // 2) * 32 + j
                i += 1
    return p


def _mask_packed(qstart, v):
    """Packed 0/1 mask [128, 768] for superblock variant v (0: qsb==0,
    1: qsb>=1), for the core whose q rows start at qstart."""
    out = np.empty((128, SEGW), np.float32)
    p = np.arange(128)[:, None]
    for (kc, off, w, qoff) in SEGS:
        i = qoff + np.arange(w)[None, :]
        kv_abs = qstart + v * QSB + kc * 128 + p - 256
        q_abs = qstart + v * QSB + i
        ok = (q_abs >= kv_abs) & (q_abs - kv_abs <= SW) & (kv_abs >= 0)
        out[:, off:off + w] = np.where(ok, 1.0, 0.0)
    return out


def _host_inputs(hidden_states, position_ids, Wq, Wk, Wv, Wo):
    hs = np.asarray(hidden_states, np.float32)
    pos = np.asarray(position_ids, np.float32)
    perm = _perm()
    wq_h = np.ascontiguousarray((np.asarray(Wq, np.float32) * SCALING)[perm].T).astype(BF16)
    wk_h = np.ascontiguousarray(np.asarray(Wk, np.float32)[perm].T).astype(BF16)
    wv_h = np.ascontiguousarray(np.asarray(Wv, np.float32).T).astype(BF16)
    wo_h = np.ascontiguousarray(np.asarray(Wo, np.float32).T).astype(BF16)
    inv = (THETA ** (-np.arange(32, dtype=np.float32) / 32.0)).astype(np.float32)
    ones8 = np.ones((128, NH), np.float32).astype(BF16)

    in_maps = []
    for c in range(N_CORES):
        b, half = c // 2, c % 2
        qstart = half * SQ
        lo = qstart - HALO
        hsT = np.zeros((HID, SKV), np.float32)
        src_lo = max(lo, 0)
        hsT[:, src_lo - lo:] = hs[b, src_lo:qstart + SQ, :].T
        pbuf = np.zeros(SKV, np.float32)
        pbuf[src_lo - lo:] = pos[b, src_lo:qstart + SQ]
        fr = inv[:, None] * pbuf[None, :]
        cosT = np.tile(np.cos(fr), (4, 1)).astype(BF16)
        sinT = np.tile(np.sin(fr), (4, 1)).astype(BF16)
        maskT = np.concatenate(
            [_mask_packed(qstart, 0), _mask_packed(qstart, 1)], axis=1)
        in_maps.append({
            "hsT": np.ascontiguousarray(hsT).astype(BF16),
            "wqT": wq_h, "wkT": wk_h, "wvT": wv_h, "woT": wo_h,
            "cosT": cosT, "sinT": sinT, "maskT": maskT.astype(BF16),
            "onesin": ones8,
        })
    return in_maps


def kernel(hidden_states, position_ids, Wq, Wk, Wv, Wo):
    global LAST_RESULT
    from concourse.bass_utils import run_bass_kernel_spmd

    if "nc" not in _cache:
        _cache["nc"] = _build_nc()
    nc = _cache["nc"]

    in_maps = _host_inputs(hidden_states, position_ids, Wq, Wk, Wv, Wo)
    trace = bool(os.environ.get("KERNEL_TRACE"))
    kw = {}
    if trace:
        kw = dict(trace=True, tmpdir=os.environ.get("KERNEL_TRACE_DIR") or None)
    res = run_bass_kernel_spmd(nc, in_maps, list(range(N_CORES)), **kw)
    LAST_RESULT = res

    out = np.empty((B, S, HID), np.float32)
    for c in range(N_CORES):
        b, half = c // 2, c % 2
        out[b, half * SQ:(half + 1) * SQ, :] = np.asarray(
            res.results[c]["out"], dtype=np.float32)
    return out
